# revision 45
# baseline (speedup 1.0000x reference)
"""Trainium2 Bass kernel for nn_MultiHeadAttention (B=2,S=128,H=16,W=16,E=256, 8 heads).

Sharding: the 512 independent (b,h,w) slices are split 64-per-core across 8
NeuronCores (pure SPMD, no collectives). Each slice is a [S=128, E=256]
self-attention problem.

Per-core pipeline (per group of 4 slices):
  - in_proj Q/K as [f,t]-transposed matmuls (bf16, moving dim = 4 slices x 128 tok)
  - in_proj V in natural [t,f] layout (X^T stationary, weights moving)
  - per slice: row-packed QK^T (K=32, 4 heads concurrent in PE), exp on ACT,
    causal mask multiply (gpsimd), row-sums + reciprocal + normalize (DVE),
    PE transposes of the attention matrix, col-packed P^T@V.
  - out_proj in fp32r (full-rate fp32 matmul for N>=256), PSUM -> DRAM via DMA.
"""

import os
import sys

import numpy as np

sys.path.insert(0, "/opt/trn_rl_repo")

from contextlib import ExitStack

import concourse.bass as bass
import concourse.mybir as mybir
import concourse.tile as tile
from concourse import bacc
from concourse.bass_utils import run_bass_kernel_spmd

P = 128
NCORES = 8
NSLICE = 64  # slices per core
GSL = 4  # slices per group
NG = NSLICE // GSL  # groups per core
NH = 8
HD = 32
E = 256
S = 128

F32 = mybir.dt.float32
F32R = mybir.dt.float32r
BF16 = mybir.dt.bfloat16
AX = mybir.AxisListType
ALU = mybir.AluOpType
AF = mybir.ActivationFunctionType


def build_program(ng=NG, repeats=1):
    """Build the SPMD Bass program (identical on all cores).

    Emission is software-pipelined: per group, the score matmuls for all 4
    slices are emitted first, then the NEXT group's in_proj, then the P^T@V
    matmuls -- so the PE always has work while the per-slice softmax chains
    (ACT/DVE/DMA-transpose) drain.
    """
    nc = bacc.Bacc("TRN2", target_bir_lowering=False, debug=False, num_devices=NCORES)

    x_d = nc.dram_tensor("x", [ng, 2, P, GSL * S], BF16, kind="ExternalInput").ap()
    wq_d = nc.dram_tensor("wq", [2, P, 256], BF16, kind="ExternalInput").ap()
    wk_d = nc.dram_tensor("wk", [2, P, 256], BF16, kind="ExternalInput").ap()
    wv_d = nc.dram_tensor("wv", [2, P, 256], BF16, kind="ExternalInput").ap()
    wo_d = nc.dram_tensor("wo", [2, P, 256], BF16, kind="ExternalInput").ap()
    am_d = nc.dram_tensor("amask", [P, S], BF16, kind="ExternalInput").ap()
    ni_d = nc.dram_tensor("negi", [P, 4 * S], BF16, kind="ExternalInput").ap()
    rm_d = nc.dram_tensor("rmask", [P, 4], F32, kind="ExternalInput").ap()
    y_d = nc.dram_tensor("y", [ng, 2, P, GSL * S], F32, kind="ExternalOutput").ap()

    with tile.TileContext(nc) as tc, ExitStack() as ctx:
        const = ctx.enter_context(tc.tile_pool(name="const", bufs=1))
        wq = const.tile([P, 2, 256], BF16, tag="wq")
        wk = const.tile([P, 2, 256], BF16, tag="wk")
        wv = const.tile([P, 2, 256], BF16, tag="wv")
        wo = const.tile([P, 2, 256], BF16, tag="wo")
        amask = const.tile([P, S], BF16, tag="amask")
        negi = const.tile([P, 4 * S], BF16, tag="negi")
        rmask = const.tile([P, 4], F32, tag="rmask")
        nc.sync.dma_start(wq[:], wq_d.rearrange("c p f -> p c f"))
        nc.sync.dma_start(wk[:], wk_d.rearrange("c p f -> p c f"))
        nc.sync.dma_start(wv[:], wv_d.rearrange("c p f -> p c f"))
        nc.sync.dma_start(wo[:], wo_d.rearrange("c p f -> p c f"))
        nc.sync.dma_start(amask[:], am_d)
        nc.sync.dma_start(negi[:], ni_d)
        nc.sync.dma_start(rmask[:], rm_d)

        xp = ctx.enter_context(tc.tile_pool(name="xp", bufs=6))
        qkp = ctx.enter_context(tc.tile_pool(name="qkp", bufs=5))
        vp = ctx.enter_context(tc.tile_pool(name="vp", bufs=6))
        pp = ctx.enter_context(tc.tile_pool(name="pp", bufs=8))
        ptp = ctx.enter_context(tc.tile_pool(name="ptp", bufs=13))
        smp = ctx.enter_context(tc.tile_pool(name="smp", bufs=12))
        redp = ctx.enter_context(tc.tile_pool(name="redp", bufs=6))
        otp = ctx.enter_context(tc.tile_pool(name="otp", bufs=3))
        ysp = ctx.enter_context(tc.tile_pool(name="ysp", bufs=3))

        # PSUM: in_proj pool, scores pool, PV-output pool, out_proj pool
        ps_big = ctx.enter_context(tc.tile_pool(name="ps_big", bufs=2, space="PSUM"))
        ps_sc = ctx.enter_context(tc.tile_pool(name="ps_sc", bufs=2, space="PSUM"))
        ps_vo = ctx.enter_context(tc.tile_pool(name="ps_vo", bufs=2, space="PSUM"))
        ps_py = ctx.enter_context(tc.tile_pool(name="ps_py", bufs=1, space="PSUM"))

        def emit_x_load(g):
            x = xp.tile([P, 2, GSL, S], BF16, tag="x")
            nc.sync.dma_start(x[:], x_d[g].rearrange("c p (sl s) -> p c sl s", sl=GSL))
            return x

        def emit_in_proj(x):
            """Returns (qt, kbd, v, chunks): six callables, each one PE-psum
            round (matmul pair + drain copies), for interleaved emission.
            K lands as block-diagonal kbd[hg][f, j, sl, l] = k[f,sl,l]*(f//32==j)
            so the per-slice QK^T is one K=128 matmul per head-group."""
            qt = qkp.tile([P, 2, GSL, S], BF16, tag="qt")
            kbd = [
                qkp.tile([P, 4, GSL, S], BF16, tag="kbd0", name="kbd0"),
                qkp.tile([P, 4, GSL, S], BF16, tag="kbd1", name="kbd1"),
            ]
            v = vp.tile([P, GSL, 256], BF16, tag="v")

            def qk_chunk(which, ft):
                wmat = (wq, wk)[which]
                ps = ps_big.tile([P, GSL * S], F32, tag="big")
                for ec in range(2):
                    nc.tensor.matmul(
                        ps[:],
                        lhsT=wmat[:, ec, ft * P : (ft + 1) * P],
                        rhs=x[:, ec].rearrange("p a b -> p (a b)"),
                        start=(ec == 0),
                        stop=(ec == 1),
                    )
                if which == 0:
                    d = qt[:, ft].rearrange("p a b -> p (a b)")
                    if ft == 0:
                        nc.vector.tensor_copy(d, ps[:])
                    else:
                        nc.scalar.copy(d, ps[:])
                else:
                    for jp in range(4):
                        d = kbd[ft][:, jp].rearrange("p a b -> p (a b)")
                        if jp < 2:
                            nc.scalar.mul(d, ps[:], rmask[:, jp : jp + 1])
                        else:
                            nc.vector.tensor_scalar(
                                d, ps[:], rmask[:, jp : jp + 1], None, ALU.mult
                            )

            def v_chunk(slp):
                psv = ps_big.tile([P, 2, 256], F32, tag="big")
                for half in range(2):
                    sl = slp * 2 + half
                    for ec in range(2):
                        nc.tensor.matmul(
                            psv[:, half],
                            lhsT=x[:, ec, sl, :],
                            rhs=wv[:, ec, :],
                            start=(ec == 0),
                            stop=(ec == 1),
                        )
                nc.vector.tensor_copy(v[:, slp * 2 : slp * 2 + 2, :], psv[:])

            chunks = [
                lambda: qk_chunk(0, 0),
                lambda: qk_chunk(0, 1),
                lambda: qk_chunk(1, 0),
                lambda: qk_chunk(1, 1),
                lambda: v_chunk(0),
                lambda: v_chunk(1),
            ]
            return qt, kbd, v, chunks

        def emit_scores_softmax(qt, kbd, sl):
            """QK^T (kbd, K=128) + mask (PE), exp (ACT), sums via add-tree +
            reduce (DVE), norm (DVE+Pool), all-head transpose (DMA xbar)."""
            pe_exp = pp.tile([P, NH, S], BF16, tag="pe")
            for hg in range(2):
                ssc = ps_sc.tile([P, 4, S], F32, tag="sc")
                nc.tensor.matmul(
                    ssc[:].rearrange("p a b -> p (a b)"),
                    lhsT=amask[:],
                    rhs=negi[:],
                    start=True,
                    stop=False,
                    skip_group_check=True,
                )
                nc.tensor.matmul(
                    ssc[:],
                    lhsT=qt[:, hg, sl, :],
                    rhs=kbd[hg][:, :, sl, :],
                    start=False,
                    stop=True,
                    skip_group_check=True,
                )
                nc.scalar.activation(
                    pe_exp[:, hg * 4 : hg * 4 + 4, :], ssc[:], AF.Exp
                )
            red1 = redp.tile([P, NH, S // 2], BF16, tag="red1")
            with nc.allow_low_precision("softmax sums tolerate bf16 rounding"):
                nc.vector.tensor_tensor(
                    red1[:], pe_exp[:, :, 0 : S // 2], pe_exp[:, :, S // 2 : S], ALU.add
                )
                red2 = redp.tile([P, NH, S // 4], BF16, tag="red2", name="red2")
                nc.vector.tensor_tensor(
                    red2[:], red1[:, :, 0 : S // 4], red1[:, :, S // 4 : S // 2], ALU.add
                )
                sums = smp.tile([P, NH], BF16, tag="sums")
                nc.vector.tensor_reduce(sums[:], red2[:], axis=AX.X, op=ALU.add)
            rcp = smp.tile([P, NH], F32, tag="rcp")
            nc.vector.reciprocal(rcp[:], sums[:])
            pn = pp.tile([P, NH, S], BF16, tag="pn")
            for i in range(NH):
                eng = nc.gpsimd if i % 2 == 0 else nc.vector
                eng.tensor_scalar(
                    pn[:, i, :], pe_exp[:, i, :], rcp[:, i : i + 1], None, ALU.mult
                )
            pt = ptp.tile([P, NH, S], BF16, tag="pt")
            nc.sync.dma_start_transpose(pt[:], pn[:].rearrange("p a b -> p (a b)"))
            return pt

        def emit_av(v, pt, py, sl):
            """PV matmuls for one slice, av copy, then this slice's column
            chunk of out_proj (no all-slices barrier)."""
            po = ps_vo.tile([P, 2, S], F32, tag="vo")
            for hg in range(2):
                for j in range(4):
                    i = hg * 4 + j
                    o32 = 32 * j
                    nc.tensor.matmul(
                        po[o32 : o32 + 32, hg, :],
                        lhsT=v[:, sl, i * 32 : (i + 1) * 32],
                        rhs=pt[:, i, :],
                        tile_position=(0, o32),
                    )
            ot = otp.tile([P, 2, S], BF16, tag="ot")
            nc.scalar.copy(ot[:], po[:])
            for et in range(2):
                for ec in range(2):
                    nc.tensor.matmul(
                        py[:, et, sl, :],
                        lhsT=wo[:, ec, et * P : (et + 1) * P],
                        rhs=ot[:, ec, :],
                        start=(ec == 0),
                        stop=(ec == 1),
                        skip_group_check=True,
                    )

        def emit_out_proj(py, g):
            y_sb = ysp.tile([P, 2, GSL * S], F32, tag="ysb")
            for et in range(2):
                if et == 0:
                    nc.vector.tensor_copy(
                        y_sb[:, et], py[:, et].rearrange("p a b -> p (a b)")
                    )
                else:
                    nc.scalar.copy(
                        y_sb[:, et], py[:, et].rearrange("p a b -> p (a b)")
                    )
            nc.gpsimd.dma_start(y_d[g].rearrange("e p f -> p e f"), y_sb[:])

        # ---- two-group-deep software pipeline
        for _rep in range(repeats):
            _emit_all(ng, emit_x_load, emit_in_proj, emit_scores_softmax,
                      emit_av, emit_out_proj, ps_py)

    nc.compile()
    return nc


def _emit_all(ng, emit_x_load, emit_in_proj, emit_scores_softmax, emit_av,
              emit_out_proj, ps_py):
    """Interleaved 3-deep software pipeline.

    Steady state, iteration g emits (round-robin so the in-order PE queue
    never head-of-line blocks on a psum-draining copy):
      scores(g+2, sl) | in_proj-chunk(g+3) | pv+out_proj(g, sl)  x4
    then leftover in_proj chunks, y(g) store, x_load(g+4).
    Scores chains get ~2 iterations of lead before their pt transposes are
    consumed by pv(g); in_proj(g+3) consumes x loaded during iteration g-1.
    """
    qkv = {}
    pts = {}
    xs = {}
    for g in range(min(4, ng)):
        xs[g] = emit_x_load(g)
    for g in range(min(3, ng)):
        qkv[g] = emit_in_proj(xs[g])
        for ch in qkv[g][3]:
            ch()
    for g in range(min(2, ng)):
        pts[g] = [emit_scores_softmax(qkv[g][0], qkv[g][1], sl) for sl in range(GSL)]
    for g in range(ng):
        sc = []
        if g + 2 < ng:
            qt2, kt2, _, _ = qkv[g + 2]
            pts[g + 2] = [None] * GSL
            sc = [(qt2, kt2, sl) for sl in range(GSL)]
        ip = []
        if g + 3 < ng:
            qkv[g + 3] = emit_in_proj(xs[g + 3])
            ip = list(qkv[g + 3][3])
        v = qkv[g][2]
        py = ps_py.tile([P, 2, GSL, S], F32, tag="py")
        for sl in range(GSL):
            if sc:
                a, b, s = sc.pop(0)
                pts[g + 2][s] = emit_scores_softmax(a, b, s)
            if ip:
                ip.pop(0)()
            emit_av(v, pts[g][sl], py, sl)
        emit_out_proj(py, g)
        for ch in ip:
            ch()
        if g + 4 < ng:
            xs[g + 4] = emit_x_load(g + 4)
        del qkv[g], pts[g]


def prep_inputs(hidden_state, w_in, w_out):
    """Host-side prep: permute weights per-head, transpose x, shard."""
    B, S_, H, W, E_ = hidden_state.shape
    nsl = B * H * W
    scale = 1.0 / np.sqrt(HD)

    idx_q = np.concatenate([np.arange(i * 96, i * 96 + 32) for i in range(NH)])
    idx_k = idx_q + 32
    idx_v = idx_q + 64
    Wq = (w_in[idx_q] * scale).astype(np.float32)  # [256 f, 256 e]
    Wk = w_in[idx_k].astype(np.float32)
    Wv = w_in[idx_v].astype(np.float32)

    def pack_w(Wm, dt):
        # lhsT layout [ec, ep, f]
        return np.ascontiguousarray(Wm.T.reshape(2, P, 256)).astype(dt)

    bf16 = np.dtype("bfloat16") if hasattr(np, "bfloat16") else None
    import ml_dtypes

    bf16 = ml_dtypes.bfloat16
    wq_h = pack_w(Wq, bf16)
    wk_h = pack_w(Wk, bf16)
    wv_h = pack_w(Wv, bf16)
    wo_h = np.ascontiguousarray(w_out.T.reshape(2, P, 256)).astype(bf16)

    am_h = np.tril(np.ones((S, S), np.float32), -1).astype(bf16)
    rm_h = np.zeros((128, 4), np.float32)
    for j in range(4):
        rm_h[32 * j : 32 * j + 32, j] = 1.0

    ni_h = np.tile(-1000.0 * np.eye(S, dtype=np.float32), (1, 4)).astype(bf16)
    ni_h = np.ascontiguousarray(
        (-1000.0 * np.eye(S, dtype=np.float32))[:, None, :].repeat(4, 1).reshape(S, 4 * S)
    ).astype(bf16)

    # x^T per slice: [slice, e, s]
    xt = hidden_state.transpose(0, 2, 3, 4, 1).reshape(nsl, E_, S_)
    xt = xt.astype(bf16)

    in_maps = []
    for c in range(NCORES):
        xs = xt[c * NSLICE : (c + 1) * NSLICE]  # [64, 256, 128]
        xs = xs.reshape(NG, GSL, 2, P, S_).transpose(0, 2, 3, 1, 4)
        xs = np.ascontiguousarray(xs.reshape(NG, 2, P, GSL * S_))
        in_maps.append(
            {
                "x": xs,
                "wq": wq_h,
                "wk": wk_h,
                "wv": wv_h,
                "wo": wo_h,
                "amask": am_h,
                "negi": ni_h,
                "rmask": rm_h,
            }
        )
    return in_maps


def assemble_output(results, B=2, H=16, W=16):
    """results: list of 8 dicts with 'y' [NG, 2, 128, GSL*S] f32."""
    ys = []
    for c in range(NCORES):
        y = results[c]["y"].reshape(NG, 2, P, GSL, S)
        y = y.transpose(0, 3, 1, 2, 4).reshape(NSLICE, E, S)
        ys.append(y)
    y_all = np.concatenate(ys, axis=0)  # [512, 256 e, 128 s]
    y_all = y_all.transpose(0, 2, 1)  # [512, s, e]
    out = y_all.reshape(B, H, W, S, E).transpose(0, 3, 1, 2, 4)
    return np.ascontiguousarray(out.astype(np.float32))


_NC_CACHE = {}


def get_program(repeats=1):
    key = repeats
    if key not in _NC_CACHE:
        _NC_CACHE[key] = build_program(repeats=repeats)
    return _NC_CACHE[key]


class _Executor:
    """Cached PJRT executor: builds the shard_map jit once, reuses across calls."""

    def __init__(self, nc):
        import jax
        from jax.sharding import Mesh, PartitionSpec
        from jax.experimental.shard_map import shard_map
        from concourse.bass2jax import _bass_exec_p, install_neuronx_cc_hook, partition_id_tensor

        install_neuronx_cc_hook()
        self.nc = nc
        pname = nc.partition_id_tensor.name if nc.partition_id_tensor else None
        in_names, out_names, out_avals, zero_outs = [], [], [], []
        for alloc in nc.m.functions[0].allocations:
            if not isinstance(alloc, mybir.MemoryLocationSet):
                continue
            name = alloc.memorylocations[0].name
            if alloc.kind == "ExternalInput":
                if name != pname:
                    in_names.append(name)
            elif alloc.kind == "ExternalOutput":
                out_names.append(name)
                shape = tuple(alloc.tensor_shape)
                dtype = mybir.dt.np(alloc.dtype)
                out_avals.append(jax.core.ShapedArray(shape, dtype))
                zero_outs.append(np.zeros(shape, dtype))
        self.in_names = in_names
        self.out_names = out_names
        self.out_avals = out_avals
        n_params = len(in_names)
        all_names = in_names + out_names + ([pname] if pname else [])

        def _body(*args):
            operands = list(args)
            if pname is not None:
                operands.append(partition_id_tensor())
            return tuple(
                _bass_exec_p.bind(
                    *operands,
                    out_avals=tuple(out_avals),
                    in_names=tuple(all_names),
                    out_names=tuple(out_names),
                    lowering_input_output_aliases=(),
                    sim_require_finite=True,
                    sim_require_nnan=True,
                    nc=nc,
                )
            )

        devices = jax.devices()[:NCORES]
        mesh = Mesh(np.asarray(devices), ("core",))
        self._mesh = mesh
        n_outs = len(out_avals)
        self._jit = jax.jit(
            shard_map(
                _body,
                mesh=mesh,
                in_specs=(PartitionSpec("core"),) * (n_params + n_outs),
                out_specs=(PartitionSpec("core"),) * n_outs,
                check_rep=False,
            ),
            keep_unused=True,
        )
        self._zero_concat = [
            np.zeros((NCORES * z.shape[0], *z.shape[1:]), z.dtype) for z in zero_outs
        ]
        self._jax = jax

    def time_body(self, in_maps, iters=8):
        """Time the device program with device-resident inputs/outputs: no
        host<->device transfers inside the timed region. Returns the list of
        per-call wall times (dispatch + device execution)."""
        import time as _time

        from jax.sharding import NamedSharding, PartitionSpec

        sh = NamedSharding(self._mesh, PartitionSpec("core"))
        concat_in = [
            np.concatenate([np.asarray(in_maps[c][nm]) for c in range(NCORES)], axis=0)
            for nm in self.in_names
        ]
        dev_in = [self._jax.device_put(a, sh) for a in concat_in]
        dev_zero = [self._jax.device_put(z, sh) for z in self._zero_concat]
        self._jax.block_until_ready(dev_in + dev_zero)
        outs = self._jit(*dev_in, *dev_zero)
        self._jax.block_until_ready(outs)  # warmup
        ts = []
        for _ in range(iters):
            t0 = _time.perf_counter()
            outs = self._jit(*dev_in, *dev_zero)
            self._jax.block_until_ready(outs)
            ts.append(_time.perf_counter() - t0)
        return ts

    def run(self, in_maps):
        concat_in = [
            np.concatenate([np.asarray(in_maps[c][nm]) for c in range(NCORES)], axis=0)
            for nm in self.in_names
        ]
        outs = self._jit(*concat_in, *self._zero_concat)
        self._jax.block_until_ready(outs)
        return [
            {
                nm: np.asarray(outs[i]).reshape(NCORES, *self.out_avals[i].shape)[c]
                for i, nm in enumerate(self.out_names)
            }
            for c in range(NCORES)
        ]


_EXEC_CACHE = {}


def get_executor(repeats=1):
    if repeats not in _EXEC_CACHE:
        _EXEC_CACHE[repeats] = _Executor(get_program(repeats))
    return _EXEC_CACHE[repeats]


def kernel(hidden_state, w_in, w_out, repeats=1):
    hidden_state = np.asarray(hidden_state, dtype=np.float32)
    w_in = np.asarray(w_in, dtype=np.float32)
    w_out = np.asarray(w_out, dtype=np.float32)
    ex = get_executor(repeats)
    in_maps = prep_inputs(hidden_state, w_in, w_out)
    results = ex.run(in_maps)
    return assemble_output(results)

